# revision 20
# baseline (speedup 1.0000x reference)
"""Trainium2 Bass kernel for a dense transformer encoder layer.

Full (unsharded) contract: kernel(**inputs) -> np.ndarray.

Model: B=4, S=2048, D=768, H=12 heads of 64, FFN 3072, two LayerNorms,
softmax attention (no mask).

Sharding: 8 cores, one (batch, query-half) pair per core — batch is data
parallel, each batch's 2048 query rows split across 2 cores. Each core
recomputes K/V for the full 2048-row sequence of its batch (cheap relative
to the rest), so there are no collectives and one SPMD program serves all
cores. Per-core inputs are row-permuted so the core's own 1024 query rows
always come first; key/value row order doesn't change attention results.

Numerics: matmuls run as float32r (TF32-style mantissa rounding, fp32
accumulate in PSUM) — 4x faster than strict fp32 on the PE at moving
free-dim >= 256. PE transposes stay exact fp32. Softmax skips the
row-max subtraction (scores here are < ~12 in magnitude, far from fp32
overflow); denominators come free from a ones-column appended to V, and
normalization is folded into the PSUM eviction of the context block.
"""

from contextlib import ExitStack

import numpy as np

import concourse.bass as bass
import concourse.tile as tile
from concourse import mybir
from concourse.bass_utils import run_bass_kernel_spmd
from concourse.masks import make_identity
from concourse.vector_clock import ScopedClock

F32 = mybir.dt.float32
F32R = mybir.dt.float32r
AF = mybir.ActivationFunctionType
ALU = mybir.AluOpType

P = 128
B, S, D, H, E, DFF = 4, 2048, 768, 12, 64, 3072
SQ = S // 2            # query rows per core
DC = D // P            # 6 d-chunks
SC = SQ // P           # 8 query-row chunks
TC = S // P            # 16 key-row chunks
WBLK = 512             # attention query-block width
NBLK = SQ // WBLK      # 2
IB = 384               # FFN intermediate block
NIB = DFF // IB        # 8
EPS = 1e-5

_MAX_WAITS = 1


class _PatchedTileContext(tile.TileContext):
    """This container's walrus rejects instructions with >1 sync wait.

    Hoist all but one wait of each committed instruction onto same-engine
    no-ops emitted just before it (sequential waits on one queue are
    equivalent to a combined wait), and split the teardown drain the same
    way.
    """

    def _split_waits(self, inst) -> None:
        si = inst.sync_info
        if si is None or not si.on_wait or len(si.on_wait) <= _MAX_WAITS:
            return
        if inst.engine == mybir.EngineType.Unassigned:
            return
        waits = list(si.on_wait)
        keep = waits[-_MAX_WAITS:]
        hoist = waits[:-_MAX_WAITS]
        for i in range(0, len(hoist), _MAX_WAITS):
            nop = mybir.InstNoOp(
                name=f"I-waitsplit-{self.nc.next_id()}",
                engine=inst.engine,
                bass_nofuse=True,
                sync_info=mybir.SyncInfo(
                    on_wait=hoist[i : i + _MAX_WAITS], on_update=[]
                ),
            )
            self._add_instruction(nop)
        inst.sync_info = mybir.SyncInfo(on_wait=keep, on_update=si.on_update)

    def _commit_instruction(self, inst, lazy_reg_writes: bool = True):
        if isinstance(inst, mybir.Instruction):
            self._split_waits(inst)
        return super()._commit_instruction(inst, lazy_reg_writes)

    def _drain_and_barrier(self, tick_clock, wait_clock):
        probe = self.nc.sync.nop(nofuse=True, hint="drain_wait_split")
        wait_clock.add_sem_waits(
            probe.ins, ScopedClock({None: tick_clock.global_clock})
        )
        self._split_waits(probe.ins)
        self.nc.sync.drain()

        self.nc.all_engine_barrier()
        assert self.sems is not None
        popped = self.nc._tile_sem_poison_stack.pop()
        assert popped is self._sem_poison
        self.nc.clear_and_free_semaphores(list(self.sems.allocated().values()))
        self.nc.all_engine_barrier()


def _layernorm(nc, sp, src, g_bc, be_bc, eps_t, out):
    """out = (src - mean)/sqrt(var+eps) * g + be, per row of [128, D]."""
    stats = sp.tile([P, 3, 6], F32, tag="ln_stats")
    for g3 in range(3):
        nc.vector.bn_stats(out=stats[:, g3, :], in_=src[:, g3 * 256 : (g3 + 1) * 256])
    mv = sp.tile([P, 2], F32, tag="ln_mv")
    nc.vector.bn_aggr(out=mv[:], in_=stats[:])
    std = sp.tile([P, 1], F32, tag="ln_std")
    nc.scalar.activation(out=std[:], in_=mv[:, 1:2], func=AF.Sqrt, bias=eps_t[:])
    rstd = sp.tile([P, 1], F32, tag="ln_rstd")
    nc.vector.reciprocal(out=rstd[:], in_=std[:])
    tmp = sp.tile([P, D], F32, tag="ln_tmp")
    nc.vector.tensor_scalar(
        out=tmp[:], in0=src[:], scalar1=mv[:, 0:1], scalar2=rstd[:],
        op0=ALU.subtract, op1=ALU.mult,
    )
    nc.vector.tensor_mul(tmp[:], tmp[:], g_bc[:])
    nc.vector.tensor_add(out[:], tmp[:], be_bc[:])


STOP_AFTER = None


def build_nc(stop_after=None):
    nc = bass.Bass("TRN2", target_bir_lowering=False, debug=False, num_devices=8)

    x = nc.dram_tensor("x", [S, D], F32, kind="ExternalInput").ap()
    wq = nc.dram_tensor("wq", [D, D], F32, kind="ExternalInput").ap()
    wk = nc.dram_tensor("wk", [D, D], F32, kind="ExternalInput").ap()
    wv = nc.dram_tensor("wv", [D, D], F32, kind="ExternalInput").ap()
    wo = nc.dram_tensor("wo", [D, D], F32, kind="ExternalInput").ap()
    w1 = nc.dram_tensor("w1", [DFF, D], F32, kind="ExternalInput").ap()
    w2 = nc.dram_tensor("w2", [D, DFF], F32, kind="ExternalInput").ap()
    b1 = nc.dram_tensor("b1", [DFF], F32, kind="ExternalInput").ap()
    b2 = nc.dram_tensor("b2", [D], F32, kind="ExternalInput").ap()
    g1 = nc.dram_tensor("g1", [D], F32, kind="ExternalInput").ap()
    be1 = nc.dram_tensor("be1", [D], F32, kind="ExternalInput").ap()
    g2 = nc.dram_tensor("g2", [D], F32, kind="ExternalInput").ap()
    be2 = nc.dram_tensor("be2", [D], F32, kind="ExternalInput").ap()
    out = nc.dram_tensor("out", [SQ, D], F32, kind="ExternalOutput").ap()

    with _PatchedTileContext(nc) as tc, ExitStack() as top:
        const = top.enter_context(tc.tile_pool(name="const", bufs=1))

        # ---- constants -------------------------------------------------
        ident = const.tile([P, P], F32)
        make_identity(nc, ident)
        identr = const.tile([P, P], F32R)
        nc.any.tensor_copy(identr[:], ident[:])
        eps_t = const.tile([P, 1], F32)
        nc.vector.memset(eps_t[:], EPS)
        ones_f = const.tile([P, 64], F32)
        nc.vector.memset(ones_f[:], 1.0)
        ones_r = const.tile([P, 64], F32R)          # rows of ones (any base)
        nc.any.tensor_copy(ones_r[:], ones_f[:])
        ones_fr = const.tile([1, P], F32)
        nc.vector.memset(ones_fr[:], 1.0)
        ones_row = const.tile([1, P], F32R)         # broadcast lhsT
        nc.any.tensor_copy(ones_row[:], ones_fr[:])

        # per-feature vectors broadcast to all 128 partitions via PE outer
        # product: bcast = ones_row.T @ vec
        vec_bc = {}
        with tc.tile_pool(name="bc_psum", bufs=2, space="PSUM") as bc_psum, \
             tc.tile_pool(name="vecstage", bufs=2) as vst:
            for name, ap in (("g1", g1), ("be1", be1), ("g2", g2),
                             ("be2", be2), ("b2", b2)):
                raw = vst.tile([1, D], F32, tag="vecraw")
                nc.sync.dma_start(out=raw[:], in_=ap[None, :])
                rr = vst.tile([1, D], F32R, tag="vecr")
                nc.any.tensor_copy(rr[:], raw[:])
                bc = const.tile([P, D], F32, tag=f"bc_{name}")
                for of in range(2):
                    ps = bc_psum.tile([P, 384], F32, tag="bcps")
                    nc.tensor.matmul(ps[:], ones_row[:], rr[:, bass.ts(of, 384)])
                    nc.any.tensor_copy(bc[:, bass.ts(of, 384)], ps[:])
                vec_bc[name] = bc

            # b1 as per-partition scalars [128, DFF//P] via PE transpose
            b1_cols = const.tile([P, DFF // P], F32)
            b1_nat = vst.tile([DFF // P, P], F32, tag="b1nat")
            nc.sync.dma_start(
                out=b1_nat[:], in_=b1.rearrange("(a p) -> a p", p=P)
            )
            psb1 = bc_psum.tile([P, DFF // P], F32, tag="b1ps")
            nc.tensor.transpose(psb1[:], b1_nat[:], ident[0 : DFF // P, 0 : DFF // P])
            nc.any.tensor_copy(b1_cols[:], psb1[:])

        # ---- long-lived activation tensors (manual pool lifetimes) -----
        kqv_pool = tc.alloc_tile_pool(name="kqv", bufs=1)
        KT = [kqv_pool.tile([P, S], F32R, tag=f"kt{i}", name=f"kt{i}") for i in range(DC)]
        QT = [kqv_pool.tile([P, SQ], F32R, tag=f"qt{i}", name=f"qt{i}") for i in range(DC)]
        VP = [kqv_pool.tile([P, H * (E + 1)], F32R, tag=f"vp{i}", name=f"vp{i}") for i in range(TC)]

        # ===============================================================
        # Phase 1+2: x load/transpose and Q/K/V projections, per seq half
        # ===============================================================
        with ExitStack() as ph:
            xnat = ph.enter_context(tc.tile_pool(name="xnat", bufs=2))
            wnat = ph.enter_context(tc.tile_pool(name="wnat", bufs=2))
            xt_pool = ph.enter_context(tc.tile_pool(name="xt", bufs=1))
            wt_pool = ph.enter_context(tc.tile_pool(name="wt", bufs=1))
            tps = ph.enter_context(tc.tile_pool(name="tps", bufs=3, space="PSUM"))
            pps = ph.enter_context(tc.tile_pool(name="pps", bufs=3, space="PSUM"))
            vps = ph.enter_context(tc.tile_pool(name="vps", bufs=2, space="PSUM"))

            def build_wt(w_ap, nm):
                """Transpose a [D, D] weight into 6 tiles [128 d, 768 f]."""
                tiles = [wt_pool.tile([P, D], F32R, tag=f"wt{d}", name=f"{nm}{d}") for d in range(DC)]
                for fc in range(DC):
                    nat = wnat.tile([P, D], F32R, tag="wnat")
                    nc.gpsimd.dma_start(out=nat[:], in_=w_ap[bass.ts(fc, P), :])
                    for dc in range(DC):
                        ps = tps.tile([P, P], F32R, tag="tp")
                        nc.tensor.transpose(ps[:], nat[:, bass.ts(dc, P)], identr[:])
                        nc.any.tensor_copy(
                            tiles[dc][:, bass.ts(fc, P)], ps[:]
                        )
                return tiles

            wqT = wkT = wvT = None
            for half in range(2):
                t0 = half * (S // 2)  # global row offset of this half
                # x rows -> xT tiles [128 d, 1024 t-local]
                xt = [xt_pool.tile([P, S // 2], F32R, tag=f"xt{d}", name=f"xth{d}") for d in range(DC)]
                for r in range(SC):
                    nat = xnat.tile([P, D], F32R, tag="xn")
                    nc.gpsimd.dma_start(
                        out=nat[:], in_=x[bass.ds(t0 + r * P, P), :]
                    )
                    for dc in range(DC):
                        ps = tps.tile([P, P], F32R, tag="tp")
                        nc.tensor.transpose(ps[:], nat[:, bass.ts(dc, P)], identr[:])
                        nc.any.tensor_copy(xt[dc][:, bass.ts(r, P)], ps[:])

                if half == 0:
                    wqT = build_wt(wq, "wqt")
                    # Q projection (half 0 only)
                    for fc in range(DC):
                        for st in range(SQ // 512):
                            ps = pps.tile([P, 512], F32, tag="pp")
                            for dc in range(DC):
                                nc.tensor.matmul(
                                    ps[:],
                                    wqT[dc][:, bass.ts(fc, P)],
                                    xt[dc][:, bass.ts(st, 512)],
                                    start=(dc == 0), stop=(dc == DC - 1),
                                )
                            nc.any.tensor_copy(
                                QT[fc][:, bass.ts(st, 512)], ps[:]
                            )

                # K projection for this half
                wkT = build_wt(wk, f"wkt{half}_")
                for fc in range(DC):
                    for st in range((S // 2) // 512):
                        ps = pps.tile([P, 512], F32, tag="pp")
                        for dc in range(DC):
                            nc.tensor.matmul(
                                ps[:],
                                wkT[dc][:, bass.ts(fc, P)],
                                xt[dc][:, bass.ts(st, 512)],
                                start=(dc == 0), stop=(dc == DC - 1),
                            )
                        nc.any.tensor_copy(
                            KT[fc][:, bass.ds(t0 + st * 512, 512)], ps[:]
                        )

                # V natural (all heads) for this half, + ones columns
                wvT = build_wt(wv, f"wvt{half}_")
                for r in range(SC):
                    tc_i = half * SC + r
                    vtile = VP[tc_i]
                    v3 = vtile.rearrange("p (h q) -> p h q", q=E + 1)
                    for of in range(2):
                        ps = vps.tile([P, 384], F32, tag="vp_ps")
                        for dc in range(DC):
                            nc.tensor.matmul(
                                ps[:],
                                xt[dc][:, bass.ts(r, P)],
                                wvT[dc][:, bass.ts(of, 384)],
                                start=(dc == 0), stop=(dc == DC - 1),
                            )
                        nc.any.tensor_copy(
                            v3[:, bass.ds(6 * of, 6), 0:E],
                            ps[:].rearrange("p (h e) -> p h e", e=E),
                        )
                    nc.any.tensor_copy(
                        v3[:, :, E : E + 1], ones_f[:, 0:H].unsqueeze(2)
                    )

        _ORDER = {None: 4, "proj": 1, "attn": 2, "wo": 3}
        _lvl = _ORDER[stop_after]
        # ===============================================================
        # Phase 3: attention — scoresT, exp, ctxT' + denominators,
        # normalize into concatT
        # ===============================================================
        if _lvl < 2:
            kqv_pool.release()
            return nc
        concat_pool = tc.alloc_tile_pool(name="concat", bufs=1, side="right")
        concatT = [concat_pool.tile([P, SQ], F32R, tag=f"cc{i}", name=f"cc{i}") for i in range(DC)]
        with ExitStack() as ph:
            expp = ph.enter_context(tc.tile_pool(name="expp", bufs=14))
            rcp = ph.enter_context(tc.tile_pool(name="rcp", bufs=2))
            tmp64 = ph.enter_context(tc.tile_pool(name="tmp64", bufs=2))
            sps = ph.enter_context(tc.tile_pool(name="sps", bufs=3, space="PSUM"))
            cps = ph.enter_context(tc.tile_pool(name="cps", bufs=3, space="PSUM"))
            bps = ph.enter_context(tc.tile_pool(name="bps", bufs=2, space="PSUM"))

            for h in range(H):
                hp, off = divmod(h, 2)
                off *= E
                for blk in range(NBLK):
                    sblk = bass.ds(blk * WBLK, WBLK)
                    expts = []
                    for t in range(TC):
                        ps_s = sps.tile([P, WBLK], F32, tag="ps_s")
                        nc.tensor.matmul(
                            ps_s[:],
                            KT[hp][off : off + E, bass.ts(t, P)],
                            QT[hp][off : off + E, sblk],
                        )
                        ex = expp.tile([P, WBLK], F32R, tag="expt")
                        nc.scalar.activation(
                            out=ex[:], in_=ps_s[:], func=AF.Exp, scale=0.125
                        )
                        expts.append(ex)

                    ps_c = cps.tile([E + 1, WBLK], F32, tag="ps_c")
                    for t in range(TC):
                        nc.tensor.matmul(
                            ps_c[:],
                            VP[t][:, h * (E + 1) : (h + 1) * (E + 1)],
                            expts[t][:],
                            start=(t == 0), stop=(t == TC - 1),
                        )

                    # reciprocal of the denominator row (partition 64)
                    rt = rcp.tile([P, WBLK], F32R, tag="recip")
                    with nc.allow_low_precision(
                        reason="fp32r-rounded softmax denominators"
                    ):
                        nc.vector.reciprocal(rt[E : E + 1, :], ps_c[E : E + 1, :])
                    # broadcast to 64 partitions via PE outer product
                    ps_b = bps.tile([E, WBLK], F32, tag="ps_b")
                    nc.tensor.matmul(
                        ps_b[:], ones_r[E : E + 1, :], rt[E : E + 1, :]
                    )
                    rbc = tmp64.tile([E, WBLK], F32R, tag="rbc")
                    nc.any.tensor_copy(rbc[:], ps_b[:])
                    # normalize + write into concatT
                    if off == 0:
                        nc.vector.tensor_mul(
                            concatT[hp][0:E, sblk], ps_c[0:E, :], rbc[:]
                        )
                    else:
                        tt = tmp64.tile([E, WBLK], F32R, tag="ctmp")
                        nc.vector.tensor_mul(tt[:], ps_c[0:E, :], rbc[:])
                        nc.sync.dma_start(
                            out=concatT[hp][E : 2 * E, sblk], in_=tt[:]
                        )

        if _lvl < 3:
            concat_pool.release()
            kqv_pool.release()
            return nc
        # ===============================================================
        # Phase 4: Wo + residual + LN1 -> h (natural) and hT
        # ===============================================================
        kqv_pool.release()
        sp = tc.alloc_tile_pool(name="scratch", bufs=2)
        hht_pool = tc.alloc_tile_pool(name="hht", bufs=1)
        h_nat = [hht_pool.tile([P, D], F32R, tag=f"h{i}", name=f"h{i}") for i in range(SC)]
        hT = [hht_pool.tile([P, SQ], F32R, tag=f"ht{i}", name=f"ht{i}") for i in range(DC)]
        with ExitStack() as ph:
            wnat = ph.enter_context(tc.tile_pool(name="wnat4", bufs=2, side="right"))
            wt_pool = ph.enter_context(tc.tile_pool(name="wt4", bufs=1, side="right"))
            xq_pool = ph.enter_context(tc.tile_pool(name="xq", bufs=1, side="right"))
            tps = ph.enter_context(tc.tile_pool(name="tps4", bufs=2, space="PSUM"))
            ops = ph.enter_context(tc.tile_pool(name="ops4", bufs=4, space="PSUM"))

            woT = [wt_pool.tile([P, D], F32R, tag=f"wot{d}", name=f"wot{d}") for d in range(DC)]
            for fc in range(DC):
                nat = wnat.tile([P, D], F32R, tag="wnat4")
                nc.gpsimd.dma_start(out=nat[:], in_=wo[bass.ts(fc, P), :])
                for dc in range(DC):
                    ps = tps.tile([P, P], F32R, tag="tp4")
                    nc.tensor.transpose(ps[:], nat[:, bass.ts(dc, P)], identr[:])
                    nc.any.tensor_copy(woT[dc][:, bass.ts(fc, P)], ps[:])

            xq = [xq_pool.tile([P, D], F32, tag=f"xq{i}", name=f"xq{i}") for i in range(SC)]
            for r in range(SC):
                nc.sync.dma_start(out=xq[r][:], in_=x[bass.ts(r, P), :])

            for r in range(SC):
                res1 = sp.tile([P, D], F32, tag="res1")
                for of in range(2):
                    ps = ops.tile([P, 384], F32, tag="wops")
                    for cc in range(DC):
                        nc.tensor.matmul(
                            ps[:],
                            concatT[cc][:, bass.ts(r, P)],
                            woT[cc][:, bass.ts(of, 384)],
                            start=(cc == 0), stop=(cc == DC - 1),
                        )
                    nc.vector.tensor_add(
                        res1[:, bass.ts(of, 384)], ps[:], xq[r][:, bass.ts(of, 384)]
                    )
                _layernorm(nc, sp, res1, vec_bc["g1"], vec_bc["be1"], eps_t, h_nat[r])
                for dc in range(DC):
                    ps = tps.tile([P, P], F32R, tag="tp4")
                    nc.tensor.transpose(ps[:], h_nat[r][:, bass.ts(dc, P)], identr[:])
                    nc.any.tensor_copy(hT[dc][:, bass.ts(r, P)], ps[:])

        if _lvl < 4:
            hht_pool.release()
            sp.release()
            concat_pool.release()
            return nc
        # ===============================================================
        # Phase 5: FFN with IB-wide intermediate blocks, SBUF accumulation
        # ===============================================================
        concat_pool.release()
        acc_pool = tc.alloc_tile_pool(name="accp", bufs=1)
        acc = [acc_pool.tile([P, D], F32, tag=f"acc{i}", name=f"acc{i}") for i in range(SC)]
        for r in range(SC):
            nc.any.tensor_copy(acc[r][:], vec_bc["b2"][:])

        with ExitStack() as ph:
            wnat = ph.enter_context(tc.tile_pool(name="wnat5", bufs=2))
            w1t_pool = ph.enter_context(tc.tile_pool(name="w1t", bufs=2))
            w2t_pool = ph.enter_context(tc.tile_pool(name="w2t", bufs=2))
            relup = ph.enter_context(tc.tile_pool(name="relu", bufs=2))
            tps = ph.enter_context(tc.tile_pool(name="tps5", bufs=2, space="PSUM"))
            f1ps = ph.enter_context(tc.tile_pool(name="f1ps", bufs=3, space="PSUM"))
            f2ps = ph.enter_context(tc.tile_pool(name="f2ps", bufs=3, space="PSUM"))

            NI = IB // P  # 3 i-chunks per block
            for ib in range(NIB):
                w1T = [w1t_pool.tile([P, IB], F32R, tag=f"w1t{d}", name=f"w1t{d}") for d in range(DC)]
                for rtile in range(NI):
                    nat = wnat.tile([P, D], F32R, tag="wnat5")
                    nc.gpsimd.dma_start(
                        out=nat[:], in_=w1[bass.ds(ib * IB + rtile * P, P), :]
                    )
                    for dc in range(DC):
                        ps = tps.tile([P, P], F32R, tag="tp5")
                        nc.tensor.transpose(ps[:], nat[:, bass.ts(dc, P)], identr[:])
                        nc.any.tensor_copy(w1T[dc][:, bass.ts(rtile, P)], ps[:])

                w2T = [w2t_pool.tile([P, D], F32R, tag=f"w2t{i}", name=f"w2t{i}") for i in range(NI)]
                for oc in range(DC):
                    nat = wnat.tile([P, IB], F32R, tag="wnat5b")
                    nc.gpsimd.dma_start(
                        out=nat[:],
                        in_=w2[bass.ts(oc, P), bass.ds(ib * IB, IB)],
                    )
                    for ic2 in range(NI):
                        ps = tps.tile([P, P], F32R, tag="tp5")
                        nc.tensor.transpose(ps[:], nat[:, bass.ts(ic2, P)], identr[:])
                        nc.any.tensor_copy(w2T[ic2][:, bass.ts(oc, P)], ps[:])

                relu1T = [relup.tile([P, SQ], F32R, tag=f"rl{i}", name=f"rl{i}") for i in range(NI)]
                for ic2 in range(NI):
                    gic = ib * NI + ic2
                    for st in range(SQ // 512):
                        ps = f1ps.tile([P, 512], F32, tag="f1")
                        for dc in range(DC):
                            nc.tensor.matmul(
                                ps[:],
                                w1T[dc][:, bass.ts(ic2, P)],
                                hT[dc][:, bass.ts(st, 512)],
                                start=(dc == 0), stop=(dc == DC - 1),
                            )
                        nc.scalar.activation(
                            out=relu1T[ic2][:, bass.ts(st, 512)], in_=ps[:],
                            func=AF.Relu, bias=b1_cols[:, gic : gic + 1],
                        )

                for r in range(SC):
                    for of in range(2):
                        ps = f2ps.tile([P, 384], F32, tag="f2")
                        for ic2 in range(NI):
                            nc.tensor.matmul(
                                ps[:],
                                relu1T[ic2][:, bass.ts(r, P)],
                                w2T[ic2][:, bass.ts(of, 384)],
                                start=(ic2 == 0), stop=(ic2 == NI - 1),
                            )
                        nc.vector.tensor_add(
                            acc[r][:, bass.ts(of, 384)], ps[:],
                            acc[r][:, bass.ts(of, 384)],
                        )

        # ===============================================================
        # Phase 6: residual + LN2 -> output
        # ===============================================================
        for r in range(SC):
            res2 = sp.tile([P, D], F32, tag="res1")
            nc.vector.tensor_add(res2[:], h_nat[r][:].bitcast(F32), acc[r][:])
            o = sp.tile([P, D], F32, tag="otile")
            _layernorm(nc, sp, res2, vec_bc["g2"], vec_bc["be2"], eps_t, o)
            nc.sync.dma_start(out=out[bass.ts(r, P), :], in_=o[:])

        acc_pool.release()
        hht_pool.release()
        sp.release()

    return nc


_CACHED = {}


def _get_nc():
    if "nc" not in _CACHED:
        _CACHED["nc"] = build_nc()
    return _CACHED["nc"]


def kernel(
    x, Wq, Wk, Wv, Wo, W1, b1, W2, b2, g1, be1, g2, be2, _trace=False, **trace_kw
):
    x = np.asarray(x, dtype=np.float32)
    shared = {
        "wq": np.ascontiguousarray(np.asarray(Wq, np.float32).reshape(D, D)),
        "wk": np.ascontiguousarray(np.asarray(Wk, np.float32).reshape(D, D)),
        "wv": np.ascontiguousarray(np.asarray(Wv, np.float32).reshape(D, D)),
        "wo": np.ascontiguousarray(np.asarray(Wo, np.float32)),
        "w1": np.ascontiguousarray(np.asarray(W1, np.float32)),
        "w2": np.ascontiguousarray(np.asarray(W2, np.float32)),
        "b1": np.asarray(b1, np.float32),
        "b2": np.asarray(b2, np.float32),
        "g1": np.asarray(g1, np.float32),
        "be1": np.asarray(be1, np.float32),
        "g2": np.asarray(g2, np.float32),
        "be2": np.asarray(be2, np.float32),
    }
    in_maps = []
    for c in range(8):
        b, half = divmod(c, 2)
        if half == 0:
            xp = x[b]
        else:
            xp = np.concatenate([x[b, SQ:], x[b, :SQ]], axis=0)
        in_maps.append({"x": np.ascontiguousarray(xp), **shared})

    nc = _get_nc()
    res = run_bass_kernel_spmd(
        nc, in_maps, core_ids=list(range(8)), trace=_trace, **trace_kw
    )
    out = np.empty((B, S, D), np.float32)
    for c in range(8):
        b, half = divmod(c, 2)
        out[b, half * SQ : (half + 1) * SQ] = res.results[c]["out"]
    if _trace:
        return out, res
    return out


# revision 26
# speedup vs baseline: 1.5692x; 1.5692x over previous
"""Trainium2 Bass kernel for a dense transformer encoder layer.

Full (unsharded) contract: kernel(**inputs) -> np.ndarray.

Model: B=4, S=2048, D=768, H=12 heads of 64, FFN 3072, two LayerNorms,
softmax attention (no mask).

Sharding: 8 cores, one (batch, query-half) pair per core — batch is data
parallel, each batch's 2048 query rows split across 2 cores. Each core
recomputes K/V for the full 2048-row sequence of its batch (cheap relative
to the rest), so there are no collectives and one SPMD program serves all
cores. Per-core inputs are row-permuted so the core's own 1024 query rows
always come first; key/value row order doesn't change attention results.

Numerics: matmuls run as float32r (TF32-style mantissa rounding, fp32
accumulate in PSUM) — 4x faster than strict fp32 on the PE at moving
free-dim >= 256. PE transposes stay exact fp32. Softmax skips the
row-max subtraction (scores here are < ~12 in magnitude, far from fp32
overflow); denominators come free from a ones-column appended to V, and
normalization is folded into the PSUM eviction of the context block.
"""

from contextlib import ExitStack

import numpy as np

import concourse.bass as bass
import concourse.tile as tile
from concourse import mybir
from concourse.bass_utils import run_bass_kernel_spmd
from concourse.masks import make_identity
from concourse.vector_clock import ScopedClock

F32 = mybir.dt.float32
F32R = mybir.dt.float32r
AF = mybir.ActivationFunctionType
ALU = mybir.AluOpType

P = 128
B, S, D, H, E, DFF = 4, 2048, 768, 12, 64, 3072
SQ = S // 2            # query rows per core
DC = D // P            # 6 d-chunks
SC = SQ // P           # 8 query-row chunks
TC = S // P            # 16 key-row chunks
WBLK = 512             # attention query-block width
NBLK = SQ // WBLK      # 2
IB = 384               # FFN intermediate block
NIB = DFF // IB        # 8
EPS = 1e-5

_MAX_WAITS = 1


class _PatchedTileContext(tile.TileContext):
    """This container's walrus rejects instructions with >1 sync wait.

    Hoist all but one wait of each committed instruction onto same-engine
    no-ops emitted just before it (sequential waits on one queue are
    equivalent to a combined wait), and split the teardown drain the same
    way.
    """

    def _split_waits(self, inst) -> None:
        si = inst.sync_info
        if si is None or not si.on_wait or len(si.on_wait) <= _MAX_WAITS:
            return
        if inst.engine == mybir.EngineType.Unassigned:
            return
        waits = list(si.on_wait)
        keep = waits[-_MAX_WAITS:]
        hoist = waits[:-_MAX_WAITS]
        for i in range(0, len(hoist), _MAX_WAITS):
            nop = mybir.InstNoOp(
                name=f"I-waitsplit-{self.nc.next_id()}",
                engine=inst.engine,
                bass_nofuse=True,
                sync_info=mybir.SyncInfo(
                    on_wait=hoist[i : i + _MAX_WAITS], on_update=[]
                ),
            )
            self._add_instruction(nop)
        inst.sync_info = mybir.SyncInfo(on_wait=keep, on_update=si.on_update)

    def _commit_instruction(self, inst, lazy_reg_writes: bool = True):
        if isinstance(inst, mybir.Instruction):
            self._split_waits(inst)
        return super()._commit_instruction(inst, lazy_reg_writes)

    def _drain_and_barrier(self, tick_clock, wait_clock):
        probe = self.nc.sync.nop(nofuse=True, hint="drain_wait_split")
        wait_clock.add_sem_waits(
            probe.ins, ScopedClock({None: tick_clock.global_clock})
        )
        self._split_waits(probe.ins)
        self.nc.sync.drain()

        self.nc.all_engine_barrier()
        assert self.sems is not None
        popped = self.nc._tile_sem_poison_stack.pop()
        assert popped is self._sem_poison
        self.nc.clear_and_free_semaphores(list(self.sems.allocated().values()))
        self.nc.all_engine_barrier()


def _layernorm(nc, sp, src, g_bc, be_bc, eps_t, out):
    """out = (src - mean)/sqrt(var+eps) * g + be, per row of [128, D]."""
    stats = sp.tile([P, 3, 6], F32, tag="ln_stats")
    for g3 in range(3):
        nc.vector.bn_stats(out=stats[:, g3, :], in_=src[:, g3 * 256 : (g3 + 1) * 256])
    mv = sp.tile([P, 2], F32, tag="ln_mv")
    nc.vector.bn_aggr(out=mv[:], in_=stats[:])
    std = sp.tile([P, 1], F32, tag="ln_std")
    nc.scalar.activation(out=std[:], in_=mv[:, 1:2], func=AF.Sqrt, bias=eps_t[:])
    rstd = sp.tile([P, 1], F32, tag="ln_rstd")
    nc.vector.reciprocal(out=rstd[:], in_=std[:])
    tmp = sp.tile([P, D], F32, tag="ln_tmp")
    nc.vector.tensor_scalar(
        out=tmp[:], in0=src[:], scalar1=mv[:, 0:1], scalar2=rstd[:],
        op0=ALU.subtract, op1=ALU.mult,
    )
    nc.vector.tensor_mul(tmp[:], tmp[:], g_bc[:])
    nc.vector.tensor_add(out[:], tmp[:], be_bc[:])


STOP_AFTER = None


def build_nc(stop_after=None):
    nc = bass.Bass("TRN2", target_bir_lowering=False, debug=False, num_devices=8)

    x = nc.dram_tensor("x", [S, D], F32, kind="ExternalInput").ap()
    wq = nc.dram_tensor("wq", [D, D], F32, kind="ExternalInput").ap()
    wk = nc.dram_tensor("wk", [D, D], F32, kind="ExternalInput").ap()
    wv = nc.dram_tensor("wv", [D, D], F32, kind="ExternalInput").ap()
    wo = nc.dram_tensor("wo", [D, D], F32, kind="ExternalInput").ap()
    w1 = nc.dram_tensor("w1", [DFF, D], F32, kind="ExternalInput").ap()
    w2 = nc.dram_tensor("w2", [D, DFF], F32, kind="ExternalInput").ap()
    b1 = nc.dram_tensor("b1", [DFF], F32, kind="ExternalInput").ap()
    b2 = nc.dram_tensor("b2", [D], F32, kind="ExternalInput").ap()
    g1 = nc.dram_tensor("g1", [D], F32, kind="ExternalInput").ap()
    be1 = nc.dram_tensor("be1", [D], F32, kind="ExternalInput").ap()
    g2 = nc.dram_tensor("g2", [D], F32, kind="ExternalInput").ap()
    be2 = nc.dram_tensor("be2", [D], F32, kind="ExternalInput").ap()
    out = nc.dram_tensor("out", [SQ, D], F32, kind="ExternalOutput").ap()

    with _PatchedTileContext(nc) as tc, ExitStack() as top:
        const = top.enter_context(tc.tile_pool(name="const", bufs=1))

        # ---- constants -------------------------------------------------
        ident = const.tile([P, P], F32)
        make_identity(nc, ident)
        identr = const.tile([P, P], F32R)
        nc.any.tensor_copy(identr[:], ident[:])
        eps_t = const.tile([P, 1], F32)
        nc.vector.memset(eps_t[:], EPS)
        ones_f = const.tile([P, 64], F32)
        nc.vector.memset(ones_f[:], 1.0)
        ones_r = const.tile([P, 64], F32R)          # rows of ones (any base)
        nc.any.tensor_copy(ones_r[:], ones_f[:])
        ones_fr = const.tile([1, P], F32)
        nc.vector.memset(ones_fr[:], 1.0)
        ones_row = const.tile([1, P], F32R)         # broadcast lhsT
        nc.any.tensor_copy(ones_row[:], ones_fr[:])

        # per-feature vectors broadcast to all 128 partitions via PE outer
        # product: bcast = ones_row.T @ vec
        vec_bc = {}
        with tc.tile_pool(name="bc_psum", bufs=2, space="PSUM") as bc_psum, \
             tc.tile_pool(name="vecstage", bufs=2) as vst:
            for name, ap in (("g1", g1), ("be1", be1), ("g2", g2),
                             ("be2", be2), ("b2", b2)):
                raw = vst.tile([1, D], F32, tag="vecraw")
                nc.sync.dma_start(out=raw[:], in_=ap[None, :])
                rr = vst.tile([1, D], F32R, tag="vecr")
                nc.any.tensor_copy(rr[:], raw[:])
                bc = const.tile([P, D], F32, tag=f"bc_{name}")
                for of in range(2):
                    ps = bc_psum.tile([P, 384], F32, tag="bcps")
                    nc.tensor.matmul(ps[:], ones_row[:], rr[:, bass.ts(of, 384)])
                    nc.any.tensor_copy(bc[:, bass.ts(of, 384)], ps[:])
                vec_bc[name] = bc

            # b1 as per-partition scalars [128, DFF//P] via PE transpose
            b1_cols = const.tile([P, DFF // P], F32)
            b1_nat = vst.tile([DFF // P, P], F32, tag="b1nat")
            nc.sync.dma_start(
                out=b1_nat[:], in_=b1.rearrange("(a p) -> a p", p=P)
            )
            psb1 = bc_psum.tile([P, DFF // P], F32, tag="b1ps")
            nc.tensor.transpose(psb1[:], b1_nat[:], ident[0 : DFF // P, 0 : DFF // P])
            nc.any.tensor_copy(b1_cols[:], psb1[:])

        # ---- long-lived activation tensors (manual pool lifetimes) -----
        kqv_pool = tc.alloc_tile_pool(name="kqv", bufs=1)
        KT = [kqv_pool.tile([P, S], F32R, tag=f"kt{i}", name=f"kt{i}") for i in range(DC)]
        QT = [kqv_pool.tile([P, SQ], F32R, tag=f"qt{i}", name=f"qt{i}") for i in range(DC)]
        VP = [kqv_pool.tile([P, H * (E + 1)], F32R, tag=f"vp{i}", name=f"vp{i}") for i in range(TC)]

        # ===============================================================
        # Phase 1+2: x load/transpose and Q/K/V projections, per seq half
        # ===============================================================
        with ExitStack() as ph:
            xnat = ph.enter_context(tc.tile_pool(name="xnat", bufs=2))
            wnat = ph.enter_context(tc.tile_pool(name="wnat", bufs=2))
            xt_pool = ph.enter_context(tc.tile_pool(name="xt", bufs=1))
            wt_pool = ph.enter_context(tc.tile_pool(name="wt", bufs=1))
            tps = ph.enter_context(tc.tile_pool(name="tps", bufs=3, space="PSUM"))
            pps = ph.enter_context(tc.tile_pool(name="pps", bufs=3, space="PSUM"))
            vps = ph.enter_context(tc.tile_pool(name="vps", bufs=2, space="PSUM"))

            def build_wt(w_ap, nm):
                """Transpose a [D, D] weight into 6 tiles [128 d, 768 f]."""
                tiles = [wt_pool.tile([P, D], F32R, tag=f"wt{d}", name=f"{nm}{d}") for d in range(DC)]
                for fc in range(DC):
                    nat = wnat.tile([P, D], F32R, tag="wnat")
                    nc.gpsimd.dma_start(out=nat[:], in_=w_ap[bass.ts(fc, P), :])
                    for dc in range(DC):
                        ps = tps.tile([P, P], F32R, tag="tp")
                        nc.tensor.transpose(ps[:], nat[:, bass.ts(dc, P)], identr[:])
                        nc.any.tensor_copy(
                            tiles[dc][:, bass.ts(fc, P)], ps[:]
                        )
                return tiles

            wqT = wkT = wvT = None
            for half in range(2):
                t0 = half * (S // 2)  # global row offset of this half
                # x rows -> xT tiles [128 d, 1024 t-local]
                xt = [xt_pool.tile([P, S // 2], F32R, tag=f"xt{d}", name=f"xth{d}") for d in range(DC)]
                for r in range(SC):
                    nat = xnat.tile([P, D], F32R, tag="xn")
                    nc.gpsimd.dma_start(
                        out=nat[:], in_=x[bass.ds(t0 + r * P, P), :]
                    )
                    for dc in range(DC):
                        ps = tps.tile([P, P], F32R, tag="tp")
                        nc.tensor.transpose(ps[:], nat[:, bass.ts(dc, P)], identr[:])
                        nc.any.tensor_copy(xt[dc][:, bass.ts(r, P)], ps[:])

                if half == 0:
                    wqT = build_wt(wq, "wqt")
                    # Q projection (half 0 only)
                    for fc in range(DC):
                        for st in range(SQ // 512):
                            ps = pps.tile([P, 512], F32, tag="pp")
                            for dc in range(DC):
                                nc.tensor.matmul(
                                    ps[:],
                                    wqT[dc][:, bass.ts(fc, P)],
                                    xt[dc][:, bass.ts(st, 512)],
                                    start=(dc == 0), stop=(dc == DC - 1),
                                )
                            nc.any.tensor_copy(
                                QT[fc][:, bass.ts(st, 512)], ps[:]
                            )

                # K projection for this half
                wkT = build_wt(wk, f"wkt{half}_")
                for fc in range(DC):
                    for st in range((S // 2) // 512):
                        ps = pps.tile([P, 512], F32, tag="pp")
                        for dc in range(DC):
                            nc.tensor.matmul(
                                ps[:],
                                wkT[dc][:, bass.ts(fc, P)],
                                xt[dc][:, bass.ts(st, 512)],
                                start=(dc == 0), stop=(dc == DC - 1),
                            )
                        nc.any.tensor_copy(
                            KT[fc][:, bass.ds(t0 + st * 512, 512)], ps[:]
                        )

                # V natural (all heads) for this half, + ones columns
                wvT = build_wt(wv, f"wvt{half}_")
                for r in range(SC):
                    tc_i = half * SC + r
                    vtile = VP[tc_i]
                    v3 = vtile.rearrange("p (h q) -> p h q", q=E + 1)
                    for of in range(2):
                        ps = vps.tile([P, 384], F32, tag="vp_ps")
                        for dc in range(DC):
                            nc.tensor.matmul(
                                ps[:],
                                xt[dc][:, bass.ts(r, P)],
                                wvT[dc][:, bass.ts(of, 384)],
                                start=(dc == 0), stop=(dc == DC - 1),
                            )
                        nc.any.tensor_copy(
                            v3[:, bass.ds(6 * of, 6), 0:E],
                            ps[:].rearrange("p (h e) -> p h e", e=E),
                        )
                    nc.any.tensor_copy(
                        v3[:, :, E : E + 1], ones_f[:, 0:H].unsqueeze(2)
                    )

        _ORDER = {None: 4, "proj": 1, "attn": 2, "wo": 3}
        _lvl = _ORDER[stop_after]
        # ===============================================================
        # Phase 3: attention — scoresT, exp, ctxT' + denominators,
        # normalize into concatT
        # ===============================================================
        if _lvl < 2:
            kqv_pool.release()
            return nc
        concat_pool = tc.alloc_tile_pool(name="concat", bufs=1, side="right")
        concatT = [concat_pool.tile([P, SQ], F32R, tag=f"cc{i}", name=f"cc{i}") for i in range(DC)]
        with ExitStack() as ph:
            expp = ph.enter_context(tc.tile_pool(name="expp", bufs=14))
            rcp = ph.enter_context(tc.tile_pool(name="rcp", bufs=2))
            tmp64 = ph.enter_context(tc.tile_pool(name="tmp64", bufs=2))
            sps = ph.enter_context(tc.tile_pool(name="sps", bufs=3, space="PSUM"))
            cps = ph.enter_context(tc.tile_pool(name="cps", bufs=3, space="PSUM"))
            bps = ph.enter_context(tc.tile_pool(name="bps", bufs=2, space="PSUM"))

            for h in range(H):
                hp, off = divmod(h, 2)
                off *= E
                for blk in range(NBLK):
                    sblk = bass.ds(blk * WBLK, WBLK)
                    expts = []
                    for t in range(TC):
                        ps_s = sps.tile([P, WBLK], F32, tag="ps_s")
                        nc.tensor.matmul(
                            ps_s[:],
                            KT[hp][off : off + E, bass.ts(t, P)],
                            QT[hp][off : off + E, sblk],
                        )
                        ex = expp.tile([P, WBLK], F32R, tag="expt")
                        nc.scalar.activation(
                            out=ex[:], in_=ps_s[:], func=AF.Exp, scale=0.125
                        )
                        expts.append(ex)

                    ps_c = cps.tile([E + 1, WBLK], F32, tag="ps_c")
                    for t in range(TC):
                        nc.tensor.matmul(
                            ps_c[:],
                            VP[t][:, h * (E + 1) : (h + 1) * (E + 1)],
                            expts[t][:],
                            start=(t == 0), stop=(t == TC - 1),
                        )

                    # reciprocal of the denominator row (partition 64)
                    rt = rcp.tile([P, WBLK], F32R, tag="recip")
                    with nc.allow_low_precision(
                        reason="fp32r-rounded softmax denominators"
                    ):
                        nc.vector.reciprocal(rt[E : E + 1, :], ps_c[E : E + 1, :])
                    # broadcast to 64 partitions via PE outer product
                    ps_b = bps.tile([E, WBLK], F32, tag="ps_b")
                    nc.tensor.matmul(
                        ps_b[:], ones_r[E : E + 1, :], rt[E : E + 1, :]
                    )
                    rbc = tmp64.tile([E, WBLK], F32R, tag="rbc")
                    nc.any.tensor_copy(rbc[:], ps_b[:])
                    # normalize + write into concatT
                    if off == 0:
                        nc.vector.tensor_mul(
                            concatT[hp][0:E, sblk], ps_c[0:E, :], rbc[:]
                        )
                    else:
                        tt = tmp64.tile([E, WBLK], F32R, tag="ctmp")
                        nc.vector.tensor_mul(tt[:], ps_c[0:E, :], rbc[:])
                        nc.sync.dma_start(
                            out=concatT[hp][E : 2 * E, sblk], in_=tt[:]
                        )

        if _lvl < 3:
            concat_pool.release()
            kqv_pool.release()
            return nc
        # ===============================================================
        # Phase 4: Wo + residual + LN1 -> h (natural) and hT
        # ===============================================================
        kqv_pool.release()
        sp = tc.alloc_tile_pool(name="scratch", bufs=2)
        hht_pool = tc.alloc_tile_pool(name="hht", bufs=1)
        h_nat = [hht_pool.tile([P, D], F32R, tag=f"h{i}", name=f"h{i}") for i in range(SC)]
        hT = [hht_pool.tile([P, SQ], F32R, tag=f"ht{i}", name=f"ht{i}") for i in range(DC)]
        with ExitStack() as ph:
            wnat = ph.enter_context(tc.tile_pool(name="wnat4", bufs=2, side="right"))
            wt_pool = ph.enter_context(tc.tile_pool(name="wt4", bufs=1, side="right"))
            xq_pool = ph.enter_context(tc.tile_pool(name="xq", bufs=1, side="right"))
            tps = ph.enter_context(tc.tile_pool(name="tps4", bufs=2, space="PSUM"))
            ops = ph.enter_context(tc.tile_pool(name="ops4", bufs=4, space="PSUM"))

            woT = [wt_pool.tile([P, D], F32R, tag=f"wot{d}", name=f"wot{d}") for d in range(DC)]
            for fc in range(DC):
                nat = wnat.tile([P, D], F32R, tag="wnat4")
                nc.gpsimd.dma_start(out=nat[:], in_=wo[bass.ts(fc, P), :])
                for dc in range(DC):
                    ps = tps.tile([P, P], F32R, tag="tp4")
                    nc.tensor.transpose(ps[:], nat[:, bass.ts(dc, P)], identr[:])
                    nc.any.tensor_copy(woT[dc][:, bass.ts(fc, P)], ps[:])

            xq = [xq_pool.tile([P, D], F32, tag=f"xq{i}", name=f"xq{i}") for i in range(SC)]
            for r in range(SC):
                nc.sync.dma_start(out=xq[r][:], in_=x[bass.ts(r, P), :])

            for r in range(SC):
                res1 = sp.tile([P, D], F32, tag="res1")
                for of in range(2):
                    ps = ops.tile([P, 384], F32, tag="wops")
                    for cc in range(DC):
                        nc.tensor.matmul(
                            ps[:],
                            concatT[cc][:, bass.ts(r, P)],
                            woT[cc][:, bass.ts(of, 384)],
                            start=(cc == 0), stop=(cc == DC - 1),
                        )
                    nc.vector.tensor_add(
                        res1[:, bass.ts(of, 384)], ps[:], xq[r][:, bass.ts(of, 384)]
                    )
                _layernorm(nc, sp, res1, vec_bc["g1"], vec_bc["be1"], eps_t, h_nat[r])
                for dc in range(DC):
                    ps = tps.tile([P, P], F32R, tag="tp4")
                    nc.tensor.transpose(ps[:], h_nat[r][:, bass.ts(dc, P)], identr[:])
                    nc.any.tensor_copy(hT[dc][:, bass.ts(r, P)], ps[:])

        if _lvl < 4:
            hht_pool.release()
            sp.release()
            concat_pool.release()
            return nc
        # ===============================================================
        # Phase 5: FFN with IB-wide intermediate blocks, SBUF accumulation
        # ===============================================================
        concat_pool.release()
        acc_pool = tc.alloc_tile_pool(name="accp", bufs=1)
        acc = [acc_pool.tile([P, D], F32, tag=f"acc{i}", name=f"acc{i}") for i in range(SC)]
        for r in range(SC):
            nc.any.tensor_copy(acc[r][:], vec_bc["b2"][:])

        with ExitStack() as ph:
            wnat = ph.enter_context(tc.tile_pool(name="wnat5", bufs=2))
            w1t_pool = ph.enter_context(tc.tile_pool(name="w1t", bufs=2))
            w2t_pool = ph.enter_context(tc.tile_pool(name="w2t", bufs=2))
            relup = ph.enter_context(tc.tile_pool(name="relu", bufs=2))
            tps = ph.enter_context(tc.tile_pool(name="tps5", bufs=2, space="PSUM"))
            f1ps = ph.enter_context(tc.tile_pool(name="f1ps", bufs=3, space="PSUM"))
            f2ps = ph.enter_context(tc.tile_pool(name="f2ps", bufs=3, space="PSUM"))

            NI = IB // P  # 3 i-chunks per block

            def build_w1(ib):
                w1T = [w1t_pool.tile([P, IB], F32R, tag=f"w1t{d}", name=f"w1t{ib}_{d}") for d in range(DC)]
                for rtile in range(NI):
                    nat = wnat.tile([P, D], F32R, tag="wnat5")
                    nc.gpsimd.dma_start(
                        out=nat[:], in_=w1[bass.ds(ib * IB + rtile * P, P), :]
                    )
                    for dc in range(DC):
                        ps = tps.tile([P, P], F32R, tag="tp5")
                        nc.tensor.transpose(ps[:], nat[:, bass.ts(dc, P)], identr[:])
                        nc.any.tensor_copy(w1T[dc][:, bass.ts(rtile, P)], ps[:])
                return w1T

            def build_w2(ib):
                w2T = [w2t_pool.tile([P, D], F32R, tag=f"w2t{i}", name=f"w2t{ib}_{i}") for i in range(NI)]
                for oc in range(DC):
                    nat = wnat.tile([P, IB], F32R, tag="wnat5b")
                    nc.gpsimd.dma_start(
                        out=nat[:],
                        in_=w2[bass.ts(oc, P), bass.ds(ib * IB, IB)],
                    )
                    for ic2 in range(NI):
                        ps = tps.tile([P, P], F32R, tag="tp5")
                        nc.tensor.transpose(ps[:], nat[:, bass.ts(ic2, P)], identr[:])
                        nc.any.tensor_copy(w2T[ic2][:, bass.ts(oc, P)], ps[:])
                return w2T

            nxt = (build_w1(0), build_w2(0))
            for ib in range(NIB):
                w1T, w2T = nxt
                relu1T = [relup.tile([P, SQ], F32R, tag=f"rl{i}", name=f"rl{ib}_{i}") for i in range(NI)]
                for ic2 in range(NI):
                    gic = ib * NI + ic2
                    for st in range(SQ // 512):
                        ps = f1ps.tile([P, 512], F32, tag="f1")
                        for dc in range(DC):
                            nc.tensor.matmul(
                                ps[:],
                                w1T[dc][:, bass.ts(ic2, P)],
                                hT[dc][:, bass.ts(st, 512)],
                                start=(dc == 0), stop=(dc == DC - 1),
                            )
                        nc.scalar.activation(
                            out=relu1T[ic2][:, bass.ts(st, 512)], in_=ps[:],
                            func=AF.Relu, bias=b1_cols[:, gic : gic + 1],
                        )

                if ib + 1 < NIB:
                    nxt = (build_w1(ib + 1), build_w2(ib + 1))

                for r in range(SC):
                    for of in range(2):
                        ps = f2ps.tile([P, 384], F32, tag="f2")
                        for ic2 in range(NI):
                            nc.tensor.matmul(
                                ps[:],
                                relu1T[ic2][:, bass.ts(r, P)],
                                w2T[ic2][:, bass.ts(of, 384)],
                                start=(ic2 == 0), stop=(ic2 == NI - 1),
                            )
                        nc.any.tensor_add(
                            acc[r][:, bass.ts(of, 384)], ps[:],
                            acc[r][:, bass.ts(of, 384)],
                        )

        # ===============================================================
        # Phase 6: residual + LN2 -> output
        # ===============================================================
        for r in range(SC):
            res2 = sp.tile([P, D], F32, tag="res1")
            nc.vector.tensor_add(res2[:], h_nat[r][:].bitcast(F32), acc[r][:])
            o = sp.tile([P, D], F32, tag="otile")
            _layernorm(nc, sp, res2, vec_bc["g2"], vec_bc["be2"], eps_t, o)
            nc.sync.dma_start(out=out[bass.ts(r, P), :], in_=o[:])

        acc_pool.release()
        hht_pool.release()
        sp.release()

    return nc


_CACHED = {}


def _get_nc():
    if "nc" not in _CACHED:
        _CACHED["nc"] = build_nc()
    return _CACHED["nc"]


def kernel(
    x, Wq, Wk, Wv, Wo, W1, b1, W2, b2, g1, be1, g2, be2, _trace=False, **trace_kw
):
    x = np.asarray(x, dtype=np.float32)
    shared = {
        "wq": np.ascontiguousarray(np.asarray(Wq, np.float32).reshape(D, D)),
        "wk": np.ascontiguousarray(np.asarray(Wk, np.float32).reshape(D, D)),
        "wv": np.ascontiguousarray(np.asarray(Wv, np.float32).reshape(D, D)),
        "wo": np.ascontiguousarray(np.asarray(Wo, np.float32)),
        "w1": np.ascontiguousarray(np.asarray(W1, np.float32)),
        "w2": np.ascontiguousarray(np.asarray(W2, np.float32)),
        "b1": np.asarray(b1, np.float32),
        "b2": np.asarray(b2, np.float32),
        "g1": np.asarray(g1, np.float32),
        "be1": np.asarray(be1, np.float32),
        "g2": np.asarray(g2, np.float32),
        "be2": np.asarray(be2, np.float32),
    }
    in_maps = []
    for c in range(8):
        b, half = divmod(c, 2)
        if half == 0:
            xp = x[b]
        else:
            xp = np.concatenate([x[b, SQ:], x[b, :SQ]], axis=0)
        in_maps.append({"x": np.ascontiguousarray(xp), **shared})

    nc = _get_nc()
    res = run_bass_kernel_spmd(
        nc, in_maps, core_ids=list(range(8)), trace=_trace, **trace_kw
    )
    out = np.empty((B, S, D), np.float32)
    for c in range(8):
        b, half = divmod(c, 2)
        out[b, half * SQ : (half + 1) * SQ] = res.results[c]["out"]
    if _trace:
        return out, res
    return out
